# revision 1
# baseline (speedup 1.0000x reference)
"""Trainium2 Bass kernel for nn_ConditionalSoftmax (sampled-softmax NLL loss).

Computes, for each batch row b:
    v_c   = vectors[cs[b]]                      # [D]
    h     = relu(v_c @ W1 + b1)                 # [H]
    logit = h @ W2 + b2                         # [V]
    nll_b = logsumexp(logit) - logit[v2s[ws[b]]]

Sharding: data-parallel over batch across 8 NeuronCores (1024 rows/core),
weights replicated.  Per core the dominant work is the [1024,512]@[512,20000]
matmul; W2 is streamed through SBUF once in bf16 (full-rate on the PE vs 1/4
rate for fp32) in 500-column tiles, logits accumulate in PSUM and are reduced
in place by the ScalarEngine's fused exp+row-sum (accum_out) so the [1024,20000]
logit matrix never touches HBM.  The target logit takes a separate cheap path:
indirect-gather of the needed W2.T rows and a fused multiply-reduce on the
VectorEngine against an fp32 recompute of h.
"""

import numpy as np
import ml_dtypes

import concourse.bass as bass
import concourse.mybir as mybir
import concourse.tile as tile
from concourse import bacc, bass_utils
from concourse.bass import IndirectOffsetOnAxis, ts
from concourse.masks import make_identity

# Problem shapes (hardcoded per contest contract)
N_VOCAB = 50000
V = 20000
D = 300
DP = 384          # D padded to 3*128
NDC = 3           # contraction chunks for D
H = 512
NHC = 4           # contraction chunks for H
B = 8192
NCORES = 8
BL = B // NCORES  # 1024 rows per core
NBT = BL // 128   # 8 batch tiles of 128 rows
VT = 1000         # vocab tile width (2 PSUM banks; bf16 matmul free-dim max 1024)
NVT = V // VT     # 20 vocab tiles

F32 = mybir.dt.float32
BF16 = mybir.dt.bfloat16
I32 = mybir.dt.int32
AF = mybir.ActivationFunctionType
OP = mybir.AluOpType

_BUILD_CACHE = {}


def _build(b1_nz: bool, b2_nz: bool):
    key = (b1_nz, b2_nz)
    if key in _BUILD_CACHE:
        return _BUILD_CACHE[key]

    nc = bacc.Bacc(
        "TRN2",
        target_bir_lowering=False,
        debug=False,
        num_devices=NCORES,
        num_swdge_queues=4,
    )

    cs_idx = nc.dram_tensor("cs_idx", [NBT, 128, 1], I32, kind="ExternalInput").ap()
    ws_idx = nc.dram_tensor("ws_idx", [NBT, 128, 1], I32, kind="ExternalInput").ap()
    vectors = nc.dram_tensor("vectors", [N_VOCAB, D], F32, kind="ExternalInput").ap()
    v2s = nc.dram_tensor("v2s", [N_VOCAB, 1], I32, kind="ExternalInput").ap()
    w1 = nc.dram_tensor("w1", [DP, H], BF16, kind="ExternalInput").ap()
    b1c = nc.dram_tensor("b1c", [NHC, 128, 1], F32, kind="ExternalInput").ap()
    w2 = nc.dram_tensor("w2", [H, V], BF16, kind="ExternalInput").ap()
    w2tb = nc.dram_tensor("w2tb", [V, H + 1], F32, kind="ExternalInput").ap()
    if b1_nz:
        b1rep = nc.dram_tensor("b1rep", [128, H], F32, kind="ExternalInput").ap()
    if b2_nz:
        b2rep = nc.dram_tensor("b2rep", [128, V], F32, kind="ExternalInput").ap()
    nll = nc.dram_tensor("nll", [NBT, 128, 1], F32, kind="ExternalOutput").ap()

    with tile.TileContext(nc) as tc:
        with (
            tc.tile_pool(name="consts", bufs=1) as consts,
            tc.tile_pool(name="idx", bufs=8) as idxp,
            tc.tile_pool(name="vc", bufs=8) as vcp,
            tc.tile_pool(name="gw", bufs=4) as gwp,
            tc.tile_pool(name="w2t", bufs=3) as w2p,
            tc.tile_pool(name="scr", bufs=2) as scrp,
            tc.tile_pool(name="ps1", bufs=2, space="PSUM") as ps1,
            tc.tile_pool(name="psmain", bufs=3, space="PSUM") as psm,
        ):
            ident = consts.tile([128, 128], BF16)
            make_identity(nc, ident[:])

            w1sb = consts.tile([128, NDC, H], BF16)
            nc.sync.dma_start(w1sb[:], w1.rearrange("(c p) h -> p c h", p=128))
            b1sb = consts.tile([128, NHC], F32)
            for hc in range(NHC):
                nc.sync.dma_start(b1sb[:, hc : hc + 1], b1c[hc])
            if b1_nz:
                b1rep_sb = consts.tile([128, H], F32)
                nc.sync.dma_start(b1rep_sb[:], b1rep[:])

            # Long-lived activations
            vcT = consts.tile([128, NDC, BL], BF16)    # v_c^T, d-major
            hT = consts.tile([128, NHC, BL], BF16)     # h^T, h-major (PE input)
            hb = consts.tile([128, NBT, H], F32)       # h, batch-major (target dot)
            sums = consts.tile([128, NBT * NVT], F32)  # per-(b,v) exp partial sums
            tdot = consts.tile([128, NBT], F32)        # target logits
            fin = consts.tile([128, 3 * NBT], F32)     # S | lnS | result

            # ---- Phase 1: gather embeddings, transpose, first layer.
            # En-masse stages (not per-tile) so each engine's queue starts
            # immediately: all index DMAs, then all gathers (parallel across
            # SWDGE queues), then casts, then PE work. ----
            cidxs, vcs, vcbs = [], [], []
            for t in range(NBT):
                cidx = idxp.tile([128, 1], I32, tag="cidx")
                nc.sync.dma_start(cidx[:], cs_idx[t])
                cidxs.append(cidx)
            for t in range(NBT):
                vc = vcp.tile([128, D], F32, tag="vc")
                nc.gpsimd.indirect_dma_start(
                    out=vc[:],
                    out_offset=None,
                    in_=vectors[:],
                    in_offset=IndirectOffsetOnAxis(ap=cidxs[t][:, :1], axis=0),
                )
                vcs.append(vc)
            for t in range(NBT):
                vcb = vcp.tile([128, DP], BF16, tag="vcb")
                nc.vector.memset(vcb[:, D:DP], 0.0)
                nc.vector.tensor_copy(vcb[:, :D], vcs[t][:])
                vcbs.append(vcb)
            for t in range(NBT):
                for c in range(NDC):
                    pt = ps1.tile([128, 128], BF16, tag="ps1")
                    nc.tensor.transpose(pt[:], vcbs[t][:, ts(c, 128)], ident[:])
                    nc.vector.tensor_copy(vcT[:, c, ts(t, 128)], pt[:])

            for t in range(NBT):
                # h^T tiles for this batch tile: [128h x 128b] per h-chunk.
                # relu+bias on DVE (keeps the ACT LUT pinned on Exp).
                for hc in range(NHC):
                    ph = ps1.tile([128, 128], F32, tag="ps1")
                    for c in range(NDC):
                        nc.tensor.matmul(
                            ph[:],
                            lhsT=w1sb[:, c, ts(hc, 128)],
                            rhs=vcT[:, c, ts(t, 128)],
                            start=(c == 0),
                            stop=(c == NDC - 1),
                        )
                    nc.vector.tensor_scalar(
                        out=hT[:, hc, ts(t, 128)],
                        in0=ph[:],
                        scalar1=b1sb[:, hc : hc + 1],
                        scalar2=0.0,
                        op0=OP.add,
                        op1=OP.max,
                    )

                # batch-major h (fp32) for the target-logit dot
                phb = ps1.tile([128, H], F32, tag="ps1")
                for c in range(NDC):
                    nc.tensor.matmul(
                        phb[:],
                        lhsT=vcT[:, c, ts(t, 128)],
                        rhs=w1sb[:, c, :],
                        start=(c == 0),
                        stop=(c == NDC - 1),
                    )
                if b1_nz:
                    nc.vector.tensor_add(phb[:], phb[:], b1rep_sb[:])
                nc.vector.tensor_scalar_max(hb[:, t, :], phb[:], 0.0)

            # ---- Phase 1b: target logit path (gathers after the cs gathers
            # so they don't delay the main-matmul critical path) ----
            for t in range(NBT):
                widx = idxp.tile([128, 1], I32, tag="widx")
                nc.sync.dma_start(widx[:], ws_idx[t])
                sidx = idxp.tile([128, 1], I32, tag="sidx")
                nc.gpsimd.indirect_dma_start(
                    out=sidx[:],
                    out_offset=None,
                    in_=v2s[:],
                    in_offset=IndirectOffsetOnAxis(ap=widx[:, :1], axis=0),
                )
                g = gwp.tile([128, H + 1], F32, tag="g")
                nc.gpsimd.indirect_dma_start(
                    out=g[:],
                    out_offset=None,
                    in_=w2tb[:],
                    in_offset=IndirectOffsetOnAxis(ap=sidx[:, :1], axis=0),
                )
                # (tensor_tensor_reduce is broken on this HW path; use 3 ops)
                gscr = gwp.tile([128, H], F32, tag="gscr")
                nc.vector.tensor_mul(gscr[:], hb[:, t, :], g[:, :H])
                gacc = gwp.tile([128, 1], F32, tag="gacc")
                nc.vector.reduce_sum(
                    out=gacc[:], in_=gscr[:], axis=mybir.AxisListType.X
                )
                nc.vector.tensor_add(tdot[:, t : t + 1], gacc[:], g[:, H : H + 1])

            # ---- Phase 2: stream W2, logits in PSUM, fused exp+rowsum ----
            w2r = w2.rearrange("(k p) v -> p k v", p=128)
            for v in range(NVT):
                w2t = w2p.tile([128, NHC, VT], BF16, tag="w2t")
                nc.sync.dma_start(w2t[:], w2r[:, :, ts(v, VT)])
                if b2_nz:
                    b2t = w2p.tile([128, VT], F32, tag="b2t")
                    nc.sync.dma_start(b2t[:], b2rep[:, ts(v, VT)])
                for t in range(NBT):
                    # [128,1000] PSUM tile spans 2 banks; a matmul cannot cross
                    # a bank boundary, so fill it as two 500-wide halves and
                    # reduce with a single 1000-wide Exp.
                    ps = psm.tile([128, VT], F32, tag="ps")
                    for k in range(NHC):
                        for lo, w in ((0, 512), (512, VT - 512)):
                            nc.tensor.matmul(
                                ps[:, lo : lo + w],
                                lhsT=hT[:, k, ts(t, 128)],
                                rhs=w2t[:, k, lo : lo + w],
                                start=(k == 0),
                                stop=(k == NHC - 1),
                            )
                    if b2_nz:
                        nc.vector.tensor_add(ps[:], ps[:], b2t[:])
                    escr = scrp.tile([128, VT], F32, tag="escr")
                    nc.scalar.activation(
                        escr[:], ps[:], AF.Exp,
                        accum_out=sums[:, t * NVT + v : t * NVT + v + 1],
                    )

            # ---- Phase 3: logsumexp and output.  Batched per op type so the
            # eight Ln's run back-to-back (one ACT table reload after Exp). ----
            for t in range(NBT):
                nc.vector.reduce_sum(
                    out=fin[:, t : t + 1],
                    in_=sums[:, ts(t, NVT)],
                    axis=mybir.AxisListType.X,
                )
            for t in range(NBT):
                nc.scalar.activation(
                    fin[:, NBT + t : NBT + t + 1], fin[:, t : t + 1], AF.Ln
                )
            for t in range(NBT):
                nc.vector.tensor_sub(
                    fin[:, 2 * NBT + t : 2 * NBT + t + 1],
                    fin[:, NBT + t : NBT + t + 1],
                    tdot[:, t : t + 1],
                )
            for t in range(NBT):
                nc.sync.dma_start(nll[t], fin[:, 2 * NBT + t : 2 * NBT + t + 1])

    nc.compile()
    _BUILD_CACHE[key] = nc
    return nc


def _prep_inputs(ws, cs, vectors, W1, b1, W2, b2, vector_to_support):
    ws = np.asarray(ws)
    cs = np.asarray(cs)
    vectors = np.asarray(vectors, dtype=np.float32)
    W1 = np.asarray(W1, dtype=np.float32)
    b1 = np.asarray(b1, dtype=np.float32)
    W2 = np.asarray(W2, dtype=np.float32)
    b2 = np.asarray(b2, dtype=np.float32)
    v2s = np.asarray(vector_to_support)

    b1_nz = bool(np.any(b1))
    b2_nz = bool(np.any(b2))

    w1p = np.zeros((DP, H), dtype=ml_dtypes.bfloat16)
    w1p[:D] = W1.astype(ml_dtypes.bfloat16)
    w2bf = np.ascontiguousarray(W2.astype(ml_dtypes.bfloat16))
    w2tb = np.ascontiguousarray(
        np.concatenate([W2.T, b2[:, None]], axis=1).astype(np.float32)
    )
    b1c = np.ascontiguousarray(b1.reshape(NHC, 128, 1))
    v2s2d = np.ascontiguousarray(v2s.astype(np.int32).reshape(N_VOCAB, 1))

    shared = {
        "vectors": np.ascontiguousarray(vectors),
        "v2s": v2s2d,
        "w1": w1p,
        "b1c": b1c,
        "w2": w2bf,
        "w2tb": w2tb,
    }
    if b1_nz:
        shared["b1rep"] = np.ascontiguousarray(
            np.broadcast_to(b1, (128, H)).astype(np.float32)
        )
    if b2_nz:
        shared["b2rep"] = np.ascontiguousarray(
            np.broadcast_to(b2, (128, V)).astype(np.float32)
        )

    in_maps = []
    for c in range(NCORES):
        sl = slice(c * BL, (c + 1) * BL)
        m = dict(shared)
        m["cs_idx"] = np.ascontiguousarray(
            cs[sl].astype(np.int32).reshape(NBT, 128, 1)
        )
        m["ws_idx"] = np.ascontiguousarray(
            ws[sl].astype(np.int32).reshape(NBT, 128, 1)
        )
        in_maps.append(m)
    return in_maps, b1_nz, b2_nz


def run(inputs: dict, trace: bool = False):
    """Run the SPMD kernel. Returns (output [B] fp32, BassKernelResults)."""
    in_maps, b1_nz, b2_nz = _prep_inputs(**inputs)
    nc = _build(b1_nz, b2_nz)
    res = bass_utils.run_bass_kernel_spmd(
        nc, in_maps, core_ids=list(range(NCORES)), trace=trace
    )
    out = np.concatenate(
        [r["nll"].reshape(-1) for r in res.results]
    ).astype(np.float32)
    return out, res


def kernel(**inputs) -> np.ndarray:
    out, _ = run(inputs, trace=False)
    return out



# revision 2
# speedup vs baseline: 1.2885x; 1.2885x over previous
"""Trainium2 Bass kernel for nn_ConditionalSoftmax (sampled-softmax NLL loss).

Computes, for each batch row b:
    v_c   = vectors[cs[b]]                      # [D]
    h     = relu(v_c @ W1 + b1)                 # [H]
    logit = h @ W2 + b2                         # [V]
    nll_b = logsumexp(logit) - logit[v2s[ws[b]]]

Sharding: data-parallel over batch across 8 NeuronCores (1024 rows/core),
weights replicated.  Per core the dominant work is the [1024,512]@[512,20000]
matmul; it runs in fp8(e4m3) DoubleRow perf mode (2 fp8 weights per PE cell,
2 MACs/cycle), with W2 pre-scaled by 16 on the host so its values sit in the
e4m3 normal range.  Logits accumulate in PSUM as [128,2048] slabs (4 banks)
and are reduced in place by the ScalarEngine's fused exp+row-sum (accum_out,
scale=1/16 undoes the W2 scaling), so the [1024,20000] logit matrix never
touches HBM.  The target logit takes a separate exact path: indirect-gather
of the needed W2.T rows (fp32) and a multiply-reduce on the VectorEngine
against an fp32 recompute of h, keeping the final NLL error ~1e-3.
"""

import numpy as np
import ml_dtypes

import concourse.bass as bass
import concourse.mybir as mybir
import concourse.tile as tile
from concourse import bacc, bass_utils
from concourse.bass import IndirectOffsetOnAxis, ts
from concourse.masks import make_identity

# Problem shapes (hardcoded per contest contract)
N_VOCAB = 50000
V = 20000
D = 300
DP = 384          # D padded to 3*128
NDC = 3           # contraction chunks for D
H = 512
NHC = 4           # contraction chunks for H (128 each)
NKP = 2           # DoubleRow contraction pairs (256 each)
B = 8192
NCORES = 8
BL = B // NCORES  # 1024 rows per core
NBT = BL // 128   # 8 batch tiles of 128 rows
W2SCALE = 16.0    # host-side scale on fp8 W2; undone by the Exp pre-scale

# Phase-2 vocab grouping: PSUM slabs of 2048 fp32 (4 banks), matmul chunks
# of <=512 so no matmul output crosses a PSUM bank. 20000 = 9*2048 + 1568.
VG = 2048
VGROUPS = [VG] * 9 + [20000 - 9 * VG]   # last = 1568
NVG = len(VGROUPS)

F32 = mybir.dt.float32
BF16 = mybir.dt.bfloat16
FP8 = mybir.dt.float8e4
I32 = mybir.dt.int32
AF = mybir.ActivationFunctionType
OP = mybir.AluOpType
DR = mybir.MatmulPerfMode.DoubleRow

_BUILD_CACHE = {}


def _chunks(width):
    """512-wide matmul chunks covering [0, width)."""
    out = []
    lo = 0
    while lo < width:
        w = min(512, width - lo)
        out.append((lo, w))
        lo += w
    return out


def _build(b1_nz: bool, b2_nz: bool):
    key = (b1_nz, b2_nz)
    if key in _BUILD_CACHE:
        return _BUILD_CACHE[key]

    nc = bacc.Bacc(
        "TRN2",
        target_bir_lowering=False,
        debug=False,
        num_devices=NCORES,
        num_swdge_queues=4,
    )

    cs_idx = nc.dram_tensor("cs_idx", [NBT, 128, 1], I32, kind="ExternalInput").ap()
    ws_idx = nc.dram_tensor("ws_idx", [NBT, 128, 1], I32, kind="ExternalInput").ap()
    vectors = nc.dram_tensor("vectors", [N_VOCAB, D], F32, kind="ExternalInput").ap()
    v2s = nc.dram_tensor("v2s", [N_VOCAB, 1], I32, kind="ExternalInput").ap()
    w1 = nc.dram_tensor("w1", [DP, H], BF16, kind="ExternalInput").ap()
    w2 = nc.dram_tensor("w2", [H, V], FP8, kind="ExternalInput").ap()
    w2tb = nc.dram_tensor("w2tb", [V, H + 1], F32, kind="ExternalInput").ap()
    if b1_nz:
        b1c = nc.dram_tensor("b1c", [NHC, 128, 1], F32, kind="ExternalInput").ap()
        b1rep = nc.dram_tensor("b1rep", [128, H], F32, kind="ExternalInput").ap()
    if b2_nz:
        b2rep = nc.dram_tensor("b2rep", [128, V], F32, kind="ExternalInput").ap()
    nll = nc.dram_tensor("nll", [NBT, 128, 1], F32, kind="ExternalOutput").ap()

    with tile.TileContext(nc) as tc:
        with (
            tc.tile_pool(name="consts", bufs=1) as consts,
            tc.tile_pool(name="idx", bufs=8) as idxp,
            tc.tile_pool(name="vc", bufs=8) as vcp,
            tc.tile_pool(name="gw", bufs=4) as gwp,
            tc.tile_pool(name="w2t", bufs=2) as w2p,
            tc.tile_pool(name="scr", bufs=2) as scrp,
            # Single PSUM pool/tag: 2 bufs x [128,2048] fp32 = all 8 banks.
            # Phase-1 tiles draw smaller shapes from the same tag.
            tc.tile_pool(name="ps", bufs=2, space="PSUM") as psp,
        ):
            ident = consts.tile([128, 128], BF16)
            make_identity(nc, ident[:])

            w1sb = consts.tile([128, NDC, H], BF16)
            nc.sync.dma_start(w1sb[:], w1.rearrange("(c p) h -> p c h", p=128))
            if b1_nz:
                b1sb = consts.tile([128, NHC], F32)
                for hc in range(NHC):
                    nc.sync.dma_start(b1sb[:, hc : hc + 1], b1c[hc])
                b1rep_sb = consts.tile([128, H], F32)
                nc.sync.dma_start(b1rep_sb[:], b1rep[:])

            # Long-lived activations
            vcT = consts.tile([128, NDC, BL], BF16)    # v_c^T, d-major
            hT = consts.tile([128, NHC, BL], FP8)      # h^T, h-major (PE fp8 input)
            hb = consts.tile([128, NBT, H], F32)       # h, batch-major (target dot)
            sums = consts.tile([128, NBT * NVG], F32)  # per-(b,vg) exp partial sums
            tdot = consts.tile([128, NBT], F32)        # target logits
            fin = consts.tile([128, 3 * NBT], F32)     # S | lnS | result

            # ---- Phase 1: gather embeddings, transpose, first layer.
            # En-masse stages (not per-tile) so each engine's queue starts
            # immediately: all index DMAs, then all gathers (parallel across
            # SWDGE queues), then casts, then PE work. ----
            cidxs, vcs, vcbs = [], [], []
            for t in range(NBT):
                cidx = idxp.tile([128, 1], I32, tag="cidx")
                nc.sync.dma_start(cidx[:], cs_idx[t])
                cidxs.append(cidx)
            for t in range(NBT):
                vc = vcp.tile([128, D], F32, tag="vc")
                nc.gpsimd.indirect_dma_start(
                    out=vc[:],
                    out_offset=None,
                    in_=vectors[:],
                    in_offset=IndirectOffsetOnAxis(ap=cidxs[t][:, :1], axis=0),
                )
                vcs.append(vc)
            for t in range(NBT):
                vcb = vcp.tile([128, DP], BF16, tag="vcb")
                nc.vector.memset(vcb[:, D:DP], 0.0)
                nc.vector.tensor_copy(vcb[:, :D], vcs[t][:])
                vcbs.append(vcb)
            for t in range(NBT):
                for c in range(NDC):
                    pt = psp.tile([128, 128], BF16, tag="ps")
                    nc.tensor.transpose(pt[:], vcbs[t][:, ts(c, 128)], ident[:])
                    nc.vector.tensor_copy(vcT[:, c, ts(t, 128)], pt[:])

            # h^T in fp8, computed in [128h x 512b] slabs; relu+bias+cast on
            # the ScalarEngine (Relu is in every ACT table set).
            for half in range(2):
                for hc in range(NHC):
                    ph = psp.tile([128, 512], F32, tag="ps")
                    for c in range(NDC):
                        nc.tensor.matmul(
                            ph[:],
                            lhsT=w1sb[:, c, ts(hc, 128)],
                            rhs=vcT[:, c, ts(half, 512)],
                            start=(c == 0),
                            stop=(c == NDC - 1),
                        )
                    nc.scalar.activation(
                        hT[:, hc, ts(half, 512)],
                        ph[:],
                        AF.Relu,
                        bias=b1sb[:, hc : hc + 1] if b1_nz else 0.0,
                    )

            # batch-major h (fp32) for the target-logit dot
            for t in range(NBT):
                phb = psp.tile([128, H], F32, tag="ps")
                for c in range(NDC):
                    nc.tensor.matmul(
                        phb[:],
                        lhsT=vcT[:, c, ts(t, 128)],
                        rhs=w1sb[:, c, :],
                        start=(c == 0),
                        stop=(c == NDC - 1),
                    )
                if b1_nz:
                    nc.vector.tensor_add(phb[:], phb[:], b1rep_sb[:])
                nc.scalar.activation(hb[:, t, :], phb[:], AF.Relu)

            # ---- Phase 1b: target logit path (gathers after the cs gathers
            # so they don't delay the main-matmul critical path) ----
            for t in range(NBT):
                widx = idxp.tile([128, 1], I32, tag="widx")
                nc.sync.dma_start(widx[:], ws_idx[t])
                sidx = idxp.tile([128, 1], I32, tag="sidx")
                nc.gpsimd.indirect_dma_start(
                    out=sidx[:],
                    out_offset=None,
                    in_=v2s[:],
                    in_offset=IndirectOffsetOnAxis(ap=widx[:, :1], axis=0),
                )
                g = gwp.tile([128, H + 1], F32, tag="g")
                nc.gpsimd.indirect_dma_start(
                    out=g[:],
                    out_offset=None,
                    in_=w2tb[:],
                    in_offset=IndirectOffsetOnAxis(ap=sidx[:, :1], axis=0),
                )
                # (tensor_tensor_reduce is broken on this HW path; use 3 ops)
                gscr = gwp.tile([128, H], F32, tag="gscr")
                nc.vector.tensor_mul(gscr[:], hb[:, t, :], g[:, :H])
                gacc = gwp.tile([128, 1], F32, tag="gacc")
                nc.vector.reduce_sum(
                    out=gacc[:], in_=gscr[:], axis=mybir.AxisListType.X
                )
                nc.vector.tensor_add(tdot[:, t : t + 1], gacc[:], g[:, H : H + 1])

            # ---- Phase 2: stream W2 (fp8), DoubleRow matmuls into [128,2048]
            # PSUM slabs, fused exp+rowsum on the ScalarEngine ----
            w2r = w2.rearrange("(c p) v -> p c v", p=128)
            v0 = 0
            for vg, vgw in enumerate(VGROUPS):
                w2t = w2p.tile([128, NHC, VG], FP8, tag="w2t")
                nc.sync.dma_start(w2t[:, :, :vgw], w2r[:, :, v0 : v0 + vgw])
                if b2_nz:
                    b2t = w2p.tile([128, VG], F32, tag="b2t")
                    nc.sync.dma_start(b2t[:, :vgw], b2rep[:, v0 : v0 + vgw])
                for t in range(NBT):
                    ps = psp.tile([128, VG], F32, tag="ps")
                    # kc-outer so the stationary hT slice is reused across
                    # the 4 column chunks of the slab.
                    for kc in range(NKP):
                        for lo, w in _chunks(vgw):
                            nc.tensor.matmul(
                                ps[:, lo : lo + w],
                                lhsT=hT[:, 2 * kc : 2 * kc + 2, ts(t, 128)],
                                rhs=w2t[:, 2 * kc : 2 * kc + 2, lo : lo + w],
                                start=(kc == 0),
                                stop=(kc == NKP - 1),
                                perf_mode=DR,
                            )
                    if b2_nz:
                        nc.vector.tensor_add(
                            ps[:, :vgw], ps[:, :vgw], b2t[:, :vgw]
                        )
                    escr = scrp.tile([128, VG], BF16, tag="escr")
                    nc.scalar.activation(
                        escr[:, :vgw], ps[:, :vgw], AF.Exp,
                        scale=1.0 / W2SCALE,
                        accum_out=sums[:, t * NVG + vg : t * NVG + vg + 1],
                    )
                v0 += vgw

            # ---- Phase 3: logsumexp and output.  Batched per op type so the
            # eight Ln's run back-to-back (one ACT table reload after Exp). ----
            for t in range(NBT):
                nc.vector.reduce_sum(
                    out=fin[:, t : t + 1],
                    in_=sums[:, ts(t, NVG)],
                    axis=mybir.AxisListType.X,
                )
            for t in range(NBT):
                nc.scalar.activation(
                    fin[:, NBT + t : NBT + t + 1], fin[:, t : t + 1], AF.Ln
                )
            for t in range(NBT):
                nc.vector.tensor_sub(
                    fin[:, 2 * NBT + t : 2 * NBT + t + 1],
                    fin[:, NBT + t : NBT + t + 1],
                    tdot[:, t : t + 1],
                )
            for t in range(NBT):
                nc.sync.dma_start(nll[t], fin[:, 2 * NBT + t : 2 * NBT + t + 1])

    nc.compile()
    _BUILD_CACHE[key] = nc
    return nc


def _prep_inputs(ws, cs, vectors, W1, b1, W2, b2, vector_to_support):
    ws = np.asarray(ws)
    cs = np.asarray(cs)
    vectors = np.asarray(vectors, dtype=np.float32)
    W1 = np.asarray(W1, dtype=np.float32)
    b1 = np.asarray(b1, dtype=np.float32)
    W2 = np.asarray(W2, dtype=np.float32)
    b2 = np.asarray(b2, dtype=np.float32)
    v2s = np.asarray(vector_to_support)

    b1_nz = bool(np.any(b1))
    b2_nz = bool(np.any(b2))

    w1p = np.zeros((DP, H), dtype=ml_dtypes.bfloat16)
    w1p[:D] = W1.astype(ml_dtypes.bfloat16)
    # fp8 W2, scaled so values land in the e4m3 normal range (TRN e4m3
    # matches OCP e4m3fn bit patterns for |x| <= 240).
    w2f8 = np.ascontiguousarray(
        np.clip(W2 * W2SCALE, -240.0, 240.0).astype(ml_dtypes.float8_e4m3fn)
    )
    w2tb = np.ascontiguousarray(
        np.concatenate([W2.T, b2[:, None]], axis=1).astype(np.float32)
    )
    v2s2d = np.ascontiguousarray(v2s.astype(np.int32).reshape(N_VOCAB, 1))

    shared = {
        "vectors": np.ascontiguousarray(vectors),
        "v2s": v2s2d,
        "w1": w1p,
        "w2": w2f8,
        "w2tb": w2tb,
    }
    if b1_nz:
        shared["b1c"] = np.ascontiguousarray(b1.reshape(NHC, 128, 1))
        shared["b1rep"] = np.ascontiguousarray(
            np.broadcast_to(b1, (128, H)).astype(np.float32)
        )
    if b2_nz:
        shared["b2rep"] = np.ascontiguousarray(
            np.broadcast_to(b2 * W2SCALE, (128, V)).astype(np.float32)
        )

    in_maps = []
    for c in range(NCORES):
        sl = slice(c * BL, (c + 1) * BL)
        m = dict(shared)
        m["cs_idx"] = np.ascontiguousarray(
            cs[sl].astype(np.int32).reshape(NBT, 128, 1)
        )
        m["ws_idx"] = np.ascontiguousarray(
            ws[sl].astype(np.int32).reshape(NBT, 128, 1)
        )
        in_maps.append(m)
    return in_maps, b1_nz, b2_nz


def run(inputs: dict, trace: bool = False):
    """Run the SPMD kernel. Returns (output [B] fp32, BassKernelResults)."""
    in_maps, b1_nz, b2_nz = _prep_inputs(**inputs)
    nc = _build(b1_nz, b2_nz)
    res = bass_utils.run_bass_kernel_spmd(
        nc, in_maps, core_ids=list(range(NCORES)), trace=trace
    )
    out = np.concatenate(
        [r["nll"].reshape(-1) for r in res.results]
    ).astype(np.float32)
    return out, res


def kernel(**inputs) -> np.ndarray:
    out, _ = run(inputs, trace=False)
    return out


# revision 3
# speedup vs baseline: 1.4358x; 1.1143x over previous
"""Trainium2 Bass kernel for nn_ConditionalSoftmax (sampled-softmax NLL loss).

Computes, for each batch row b:
    v_c   = vectors[cs[b]]                      # [D]
    h     = relu(v_c @ W1 + b1)                 # [H]
    logit = h @ W2 + b2                         # [V]
    nll_b = logsumexp(logit) - logit[v2s[ws[b]]]

Sharding: data-parallel over batch across 8 NeuronCores (1024 rows/core),
weights replicated.  Per core the dominant work is the [1024,512]@[512,20000]
matmul; it runs in fp8(e4m3) DoubleRow perf mode (2 fp8 weights per PE cell,
2 MACs/cycle), with W2 pre-scaled by 16 on the host so its values sit in the
e4m3 normal range.  W2 (10 MB in fp8) is preloaded once into SBUF and stays
resident; phase 2 then runs batch-tile-outer with zero DMA.  Logits accumulate
in PSUM as [128,2048] slabs (4 banks) and are reduced by the ScalarEngine's
fused exp+row-sum (accum_out, scale=1/16 undoes the W2 scaling) written back
in place over PSUM (avoids the ACT SBUF write bubble), so the [1024,20000]
logit matrix never touches HBM.  The target logit takes a separate exact
path: indirect-gather of the needed W2.T rows (fp32) and a multiply-reduce on
the VectorEngine against an fp32 recompute of h, keeping NLL error ~1e-3.
"""

import numpy as np
import ml_dtypes

import concourse.bass as bass
import concourse.mybir as mybir
import concourse.tile as tile
from concourse import bacc, bass_utils
from concourse.bass import IndirectOffsetOnAxis, ts
from concourse.masks import make_identity

# Problem shapes (hardcoded per contest contract)
N_VOCAB = 50000
V = 20000
D = 300
DP = 384          # D padded to 3*128
NDC = 3           # contraction chunks for D
H = 512
NHC = 4           # contraction chunks for H (128 each)
NKP = 2           # DoubleRow contraction pairs (256 each)
B = 8192
NCORES = 8
BL = B // NCORES  # 1024 rows per core
NBT = BL // 128   # 8 batch tiles of 128 rows
W2SCALE = 16.0    # host-side scale on fp8 W2; undone by the Exp pre-scale

# Phase-2 vocab grouping: PSUM slabs of 2048 fp32 (4 banks), matmul chunks
# of <=512 so no matmul output crosses a PSUM bank. 20000 = 9*2048 + 1568.
VG = 2048
VGROUPS = [VG] * 9 + [20000 - 9 * VG]   # last = 1568
NVG = len(VGROUPS)
NW2DMA = 20       # W2 preload split for DMA-queue parallelism

F32 = mybir.dt.float32
BF16 = mybir.dt.bfloat16
FP8 = mybir.dt.float8e4
I32 = mybir.dt.int32
AF = mybir.ActivationFunctionType
OP = mybir.AluOpType
DR = mybir.MatmulPerfMode.DoubleRow

_BUILD_CACHE = {}


def _chunks(width):
    """512-wide matmul chunks covering [0, width)."""
    out = []
    lo = 0
    while lo < width:
        w = min(512, width - lo)
        out.append((lo, w))
        lo += w
    return out


def _build(b1_nz: bool, b2_nz: bool):
    key = (b1_nz, b2_nz)
    if key in _BUILD_CACHE:
        return _BUILD_CACHE[key]

    nc = bacc.Bacc(
        "TRN2",
        target_bir_lowering=False,
        debug=False,
        num_devices=NCORES,
        num_swdge_queues=4,
    )

    cs_idx = nc.dram_tensor("cs_idx", [NBT, 128, 1], I32, kind="ExternalInput").ap()
    ws_idx = nc.dram_tensor("ws_idx", [NBT, 128, 1], I32, kind="ExternalInput").ap()
    vectors = nc.dram_tensor("vectors", [N_VOCAB, D], F32, kind="ExternalInput").ap()
    v2s = nc.dram_tensor("v2s", [N_VOCAB, 1], I32, kind="ExternalInput").ap()
    w1 = nc.dram_tensor("w1", [DP, H], BF16, kind="ExternalInput").ap()
    w2 = nc.dram_tensor("w2", [H, V], FP8, kind="ExternalInput").ap()
    w2tb = nc.dram_tensor("w2tb", [V, H + 1], F32, kind="ExternalInput").ap()
    if b1_nz:
        b1c = nc.dram_tensor("b1c", [NHC, 128, 1], F32, kind="ExternalInput").ap()
        b1rep = nc.dram_tensor("b1rep", [128, H], F32, kind="ExternalInput").ap()
    if b2_nz:
        b2rep = nc.dram_tensor("b2rep", [128, V], F32, kind="ExternalInput").ap()
    nll = nc.dram_tensor("nll", [NBT, 128, 1], F32, kind="ExternalOutput").ap()

    with tile.TileContext(nc) as tc:
        with (
            tc.tile_pool(name="consts", bufs=1) as consts,
            tc.tile_pool(name="idx", bufs=8) as idxp,
            tc.tile_pool(name="vc", bufs=8) as vcp,
            tc.tile_pool(name="gw", bufs=4) as gwp,
            # Single PSUM pool/tag: 2 bufs x [128,2048] fp32 = all 8 banks.
            # Phase-1 tiles draw smaller shapes from the same tag.
            tc.tile_pool(name="ps", bufs=2, space="PSUM") as psp,
        ):
            ident = consts.tile([128, 128], BF16)
            make_identity(nc, ident[:])

            w1sb = consts.tile([128, NDC, H], BF16)
            nc.sync.dma_start(w1sb[:], w1.rearrange("(c p) h -> p c h", p=128))
            if b1_nz:
                b1sb = consts.tile([128, NHC], F32)
                for hc in range(NHC):
                    nc.sync.dma_start(b1sb[:, hc : hc + 1], b1c[hc])
                b1rep_sb = consts.tile([128, H], F32)
                nc.sync.dma_start(b1rep_sb[:], b1rep[:])

            # Long-lived activations / resident weights
            w2sb = consts.tile([128, NHC, V], FP8)     # all of W2, resident
            vcT = consts.tile([128, NDC, BL], BF16)    # v_c^T, d-major
            hT = consts.tile([128, NHC, BL], FP8)      # h^T, h-major (PE fp8 input)
            hb = consts.tile([128, NBT, H], F32)       # h, batch-major (target dot)
            sums = consts.tile([128, NBT * NVG], F32)  # per-(b,vg) exp partial sums
            tdot = consts.tile([128, NBT], F32)        # target logits
            fin = consts.tile([128, 3 * NBT], F32)     # S | lnS | result
            if b2_nz:
                b2sb = consts.tile([128, V], F32)
                nc.sync.dma_start(b2sb[:], b2rep[:])

            # ---- Phase 1a: gather embeddings (en-masse so DMA queues fill
            # immediately), cast to bf16. ----
            cidxs, vcs, vcbs = [], [], []
            for t in range(NBT):
                cidx = idxp.tile([128, 1], I32, tag="cidx")
                nc.sync.dma_start(cidx[:], cs_idx[t])
                cidxs.append(cidx)
            for t in range(NBT):
                vc = vcp.tile([128, D], F32, tag="vc")
                nc.gpsimd.indirect_dma_start(
                    out=vc[:],
                    out_offset=None,
                    in_=vectors[:],
                    in_offset=IndirectOffsetOnAxis(ap=cidxs[t][:, :1], axis=0),
                )
                vcs.append(vc)

            # W2 preload, split into v-chunks for DMA-queue parallelism.
            # Emitted after the gathers so they win the queues first.
            w2r = w2.rearrange("(c p) v -> p c v", p=128)
            wv = V // NW2DMA
            for i in range(NW2DMA):
                nc.sync.dma_start(
                    w2sb[:, :, ts(i, wv)], w2r[:, :, ts(i, wv)]
                )

            for t in range(NBT):
                vcb = vcp.tile([128, DP], BF16, tag="vcb")
                nc.vector.memset(vcb[:, D:DP], 0.0)
                nc.vector.tensor_copy(vcb[:, :D], vcs[t][:])
                vcbs.append(vcb)

            def phase1a_half(half):
                # transposes + first layer for batch tiles 4*half..4*half+3;
                # h^T in fp8 slabs [128h x 512b]; relu+bias+cast on the DVE.
                for tt in range(4):
                    t = 4 * half + tt
                    for c in range(NDC):
                        pt = psp.tile([128, 128], BF16, tag="ps")
                        nc.tensor.transpose(
                            pt[:], vcbs[t][:, ts(c, 128)], ident[:]
                        )
                        nc.vector.tensor_copy(vcT[:, c, ts(t, 128)], pt[:])
                for hc in range(NHC):
                    ph = psp.tile([128, 512], F32, tag="ps")
                    for c in range(NDC):
                        nc.tensor.matmul(
                            ph[:],
                            lhsT=w1sb[:, c, ts(hc, 128)],
                            rhs=vcT[:, c, ts(half, 512)],
                            start=(c == 0),
                            stop=(c == NDC - 1),
                        )
                    if b1_nz:
                        nc.vector.tensor_scalar(
                            out=hT[:, hc, ts(half, 512)],
                            in0=ph[:],
                            scalar1=b1sb[:, hc : hc + 1],
                            scalar2=0.0,
                            op0=OP.add,
                            op1=OP.max,
                        )
                    else:
                        nc.vector.tensor_scalar_max(
                            hT[:, hc, ts(half, 512)], ph[:], 0.0
                        )

            def phase2_t(t):
                # DoubleRow matmuls into [128,2048] PSUM slabs from resident
                # W2; fused exp+rowsum in place on the ScalarEngine.
                v0 = 0
                for vg, vgw in enumerate(VGROUPS):
                    ps = psp.tile([128, VG], F32, tag="ps")
                    # kc-outer so the stationary hT slice is reused across
                    # the 4 column chunks of the slab.
                    for kc in range(NKP):
                        for lo, w in _chunks(vgw):
                            nc.tensor.matmul(
                                ps[:, lo : lo + w],
                                lhsT=hT[:, 2 * kc : 2 * kc + 2, ts(t, 128)],
                                rhs=w2sb[:, 2 * kc : 2 * kc + 2, v0 + lo : v0 + lo + w],
                                start=(kc == 0),
                                stop=(kc == NKP - 1),
                                perf_mode=DR,
                            )
                    if b2_nz:
                        nc.vector.tensor_add(
                            ps[:, :vgw], ps[:, :vgw], b2sb[:, v0 : v0 + vgw]
                        )
                    nc.scalar.activation(
                        ps[:, :vgw], ps[:, :vgw], AF.Exp,
                        scale=1.0 / W2SCALE,
                        accum_out=sums[:, t * NVG + vg : t * NVG + vg + 1],
                    )
                    v0 += vgw

            def phase1b_t(t):
                # target logit path: exact fp32 h recompute + gathered W2.T row
                widx = idxp.tile([128, 1], I32, tag="widx")
                nc.sync.dma_start(widx[:], ws_idx[t])
                sidx = idxp.tile([128, 1], I32, tag="sidx")
                nc.gpsimd.indirect_dma_start(
                    out=sidx[:],
                    out_offset=None,
                    in_=v2s[:],
                    in_offset=IndirectOffsetOnAxis(ap=widx[:, :1], axis=0),
                )
                g = gwp.tile([128, H + 1], F32, tag="g")
                nc.gpsimd.indirect_dma_start(
                    out=g[:],
                    out_offset=None,
                    in_=w2tb[:],
                    in_offset=IndirectOffsetOnAxis(ap=sidx[:, :1], axis=0),
                )
                phb = psp.tile([128, H], F32, tag="ps")
                for c in range(NDC):
                    nc.tensor.matmul(
                        phb[:],
                        lhsT=vcT[:, c, ts(t, 128)],
                        rhs=w1sb[:, c, :],
                        start=(c == 0),
                        stop=(c == NDC - 1),
                    )
                if b1_nz:
                    nc.vector.tensor_add(phb[:], phb[:], b1rep_sb[:])
                nc.vector.tensor_scalar_max(hb[:, t, :], phb[:], 0.0)
                # (tensor_tensor_reduce is broken on this HW path; use 3 ops)
                gscr = gwp.tile([128, H], F32, tag="gscr")
                nc.vector.tensor_mul(gscr[:], hb[:, t, :], g[:, :H])
                gacc = gwp.tile([128, 1], F32, tag="gacc")
                nc.vector.reduce_sum(
                    out=gacc[:], in_=gscr[:], axis=mybir.AxisListType.X
                )
                nc.vector.tensor_add(tdot[:, t : t + 1], gacc[:], g[:, H : H + 1])

            # Interleaved emission: PE starts phase 2 for the first half's
            # batch tiles while the second half's phase 1 and every tile's
            # target-gather path overlap it.
            phase1a_half(0)
            for t in range(NBT):
                if t == 1:
                    phase1a_half(1)
                phase2_t(t)
                phase1b_t(t)

            # ---- Phase 3: logsumexp and output.  Batched per op type so the
            # eight Ln's run back-to-back (one ACT table reload after Exp). ----
            for t in range(NBT):
                nc.vector.reduce_sum(
                    out=fin[:, t : t + 1],
                    in_=sums[:, ts(t, NVG)],
                    axis=mybir.AxisListType.X,
                )
            for t in range(NBT):
                nc.scalar.activation(
                    fin[:, NBT + t : NBT + t + 1], fin[:, t : t + 1], AF.Ln
                )
            for t in range(NBT):
                nc.vector.tensor_sub(
                    fin[:, 2 * NBT + t : 2 * NBT + t + 1],
                    fin[:, NBT + t : NBT + t + 1],
                    tdot[:, t : t + 1],
                )
            for t in range(NBT):
                nc.sync.dma_start(nll[t], fin[:, 2 * NBT + t : 2 * NBT + t + 1])

    nc.compile()
    _BUILD_CACHE[key] = nc
    return nc


def _prep_inputs(ws, cs, vectors, W1, b1, W2, b2, vector_to_support):
    ws = np.asarray(ws)
    cs = np.asarray(cs)
    vectors = np.asarray(vectors, dtype=np.float32)
    W1 = np.asarray(W1, dtype=np.float32)
    b1 = np.asarray(b1, dtype=np.float32)
    W2 = np.asarray(W2, dtype=np.float32)
    b2 = np.asarray(b2, dtype=np.float32)
    v2s = np.asarray(vector_to_support)

    b1_nz = bool(np.any(b1))
    b2_nz = bool(np.any(b2))

    w1p = np.zeros((DP, H), dtype=ml_dtypes.bfloat16)
    w1p[:D] = W1.astype(ml_dtypes.bfloat16)
    # fp8 W2, scaled so values land in the e4m3 normal range (TRN e4m3
    # matches OCP e4m3fn bit patterns for |x| <= 240).
    w2f8 = np.ascontiguousarray(
        np.clip(W2 * W2SCALE, -240.0, 240.0).astype(ml_dtypes.float8_e4m3fn)
    )
    w2tb = np.ascontiguousarray(
        np.concatenate([W2.T, b2[:, None]], axis=1).astype(np.float32)
    )
    v2s2d = np.ascontiguousarray(v2s.astype(np.int32).reshape(N_VOCAB, 1))

    shared = {
        "vectors": np.ascontiguousarray(vectors),
        "v2s": v2s2d,
        "w1": w1p,
        "w2": w2f8,
        "w2tb": w2tb,
    }
    if b1_nz:
        shared["b1c"] = np.ascontiguousarray(b1.reshape(NHC, 128, 1))
        shared["b1rep"] = np.ascontiguousarray(
            np.broadcast_to(b1, (128, H)).astype(np.float32)
        )
    if b2_nz:
        shared["b2rep"] = np.ascontiguousarray(
            np.broadcast_to(b2 * W2SCALE, (128, V)).astype(np.float32)
        )

    in_maps = []
    for c in range(NCORES):
        sl = slice(c * BL, (c + 1) * BL)
        m = dict(shared)
        m["cs_idx"] = np.ascontiguousarray(
            cs[sl].astype(np.int32).reshape(NBT, 128, 1)
        )
        m["ws_idx"] = np.ascontiguousarray(
            ws[sl].astype(np.int32).reshape(NBT, 128, 1)
        )
        in_maps.append(m)
    return in_maps, b1_nz, b2_nz


def run(inputs: dict, trace: bool = False):
    """Run the SPMD kernel. Returns (output [B] fp32, BassKernelResults)."""
    in_maps, b1_nz, b2_nz = _prep_inputs(**inputs)
    nc = _build(b1_nz, b2_nz)
    res = bass_utils.run_bass_kernel_spmd(
        nc, in_maps, core_ids=list(range(NCORES)), trace=trace
    )
    out = np.concatenate(
        [r["nll"].reshape(-1) for r in res.results]
    ).astype(np.float32)
    return out, res


def kernel(**inputs) -> np.ndarray:
    out, _ = run(inputs, trace=False)
    return out
